# revision 11
# baseline (speedup 1.0000x reference)
"""Trainium2 Bass kernel for nn_CausalSelfAttention (B=1, T=2048, D=1024, H=16).

Sharding: 2 heads per core across 8 cores (tensor parallel). Wq/Wk/Wv
column-sharded by head, attention fully local, Wo row-sharded; host sums the
8 partial outputs (the all-reduce of the unshard step).

v3: bf16 matmul datapaths; rope/stats elementwise batched per 4-tile group
over persistent SBUF buffers (fewer, bigger DVE/GPSIMD instructions);
v-blend on DVE direct from PSUM; ones columns initialized once; longer HAM
warmup so the QKV phase starts at 2.4 GHz.

Per-core pipeline:
  P1  fused QKV: psum[t,384] = sum_i xT_blk.T @ [WqT|WkT|(1-l)WvT]; evac q,k
      (bf16) to qkall, v-blend (+lam*vi) from PSUM into vball.
  P2  per 4-tile group: RMS stats (Square+reduce), rsqrt bit-trick, RoPE via
      concat-table trick with sliced swap-reads, scale -> qkrall.
  P3  PE-transpose roped q,k -> qT,kT (d-major, bf16).
  P4  per (ci, head): ST[tk,tq] = kT_slice.T @ qT_chunk into 2-bank PSUM duos,
      one Exp per duo (bf16 out), tri-mask on diagonal blocks (DVE), matmul2
      YT[d|L,tq] with lhsT=[v|1] and rhs=E, e^sink accumulated via K=1 matmul
      so scale = sigmoid(lse-sink)/L = 1/(L + e^sink).
  P5  broadcast 1/(L+e^sink) across partitions via K=1 matmul + reciprocal,
      scale YT -> yts (bf16), out-proj per head (K=128), evacuate bf16, DMA.
"""

import sys

if "/opt/trn_rl_repo" not in sys.path:
    sys.path.insert(0, "/opt/trn_rl_repo")

import numpy as np
import ml_dtypes
from contextlib import ExitStack

from concourse import bacc, tile
from concourse import mybir
from concourse.bass_utils import run_bass_kernel_spmd

F32 = mybir.dt.float32
F32R = mybir.dt.float32r
BF16 = mybir.dt.bfloat16
I32 = mybir.dt.int32
AF = mybir.ActivationFunctionType
ALU = mybir.AluOpType
AX = mybir.AxisListType

T = 2048
D = 1024
HD = 64
NT = T // 128  # 16 t-tiles
RMS_EPS = float(np.finfo(np.float32).eps)
BF = ml_dtypes.bfloat16


def _build_program():
    nc = bacc.Bacc("TRN2", target_bir_lowering=False, debug=False, num_devices=8)

    d_xtb = nc.dram_tensor("xtb", [NT, 128, 8, 128], BF16, kind="ExternalInput").ap()
    d_wqkv = nc.dram_tensor("wqkv", [128, 8, 384], BF16, kind="ExternalInput").ap()
    d_vis = nc.dram_tensor("vis", [128, NT, 128], BF16, kind="ExternalInput").ap()
    d_cc = nc.dram_tensor("cc", [128, NT, 64], BF16, kind="ExternalInput").ap()
    d_sc = nc.dram_tensor("sc", [128, NT, 64], BF16, kind="ExternalInput").ap()
    d_wo = nc.dram_tensor("wo", [128, D], BF16, kind="ExternalInput").ap()
    d_idn = nc.dram_tensor("idn", [128, 128], BF16, kind="ExternalInput").ap()
    d_tri = nc.dram_tensor("tri", [128, 128], BF16, kind="ExternalInput").ap()
    d_onp = nc.dram_tensor("onp", [66, 128], F32R, kind="ExternalInput").ap()
    d_lsbi = nc.dram_tensor("lsbi", [1, 4096], F32R, kind="ExternalInput").ap()
    d_out = nc.dram_tensor("out", [D, T], BF16, kind="ExternalOutput").ap()

    with tile.TileContext(nc) as tc, ExitStack() as ctx:
        sb = ctx.enter_context(tc.tile_pool(name="sb", bufs=1))
        sb_x = ctx.enter_context(tc.tile_pool(name="sb_x", bufs=4))
        sb_w1 = ctx.enter_context(tc.tile_pool(name="sb_w1", bufs=3))
        sb_w2 = ctx.enter_context(tc.tile_pool(name="sb_w2", bufs=3))
        sb_e = ctx.enter_context(tc.tile_pool(name="sb_e", bufs=3))
        sb_o = ctx.enter_context(tc.tile_pool(name="sb_o", bufs=3))
        ps = ctx.enter_context(tc.tile_pool(name="ps", bufs=2, space="PSUM"))

        # weights first on the sync queue (needed by the first matmul);
        # other constants go via the gpsimd queue so they don't delay x.
        # Split so the first two k-chunks land before x tile 0, the rest after.
        wqkv = sb.tile([128, 8, 384], BF16)
        nc.sync.dma_start(out=wqkv[:, 0:2, :], in_=d_wqkv[:, 0:2, :])
        wqkv_rest = [None]
        vi_t = sb.tile([128, NT, 128], BF16)
        cc_t = sb.tile([128, NT, 64], BF16)
        sc_t = sb.tile([128, NT, 64], BF16)
        wo = sb.tile([128, D], BF16)
        const_dmas = []
        early_dmas = []
        early_dmas.append(nc.gpsimd.dma_start(out=vi_t[:], in_=d_vis[:]))
        early_dmas.append(nc.gpsimd.dma_start(out=cc_t[:], in_=d_cc[:]))
        early_dmas.append(nc.gpsimd.dma_start(out=sc_t[:], in_=d_sc[:]))
        const_dmas.append(nc.gpsimd.dma_start(out=wo[:], in_=d_wo[:]))
        idn = sb.tile([128, 128], BF16)
        early_dmas.append(nc.gpsimd.dma_start(out=idn[:], in_=d_idn[:]))
        tri = sb.tile([128, 128], BF16)
        const_dmas.append(nc.gpsimd.dma_start(out=tri[:], in_=d_tri[:]))
        onp = sb.tile([66, 128], F32R)
        const_dmas.append(nc.gpsimd.dma_start(out=onp[:], in_=d_onp[:]))

        stats = sb.tile([128, 64], F32)
        rbuf = sb.tile([128, 64], F32)
        rbufb = sb.tile([128, 64], BF16)
        qT = sb.tile([128, T], BF16)
        kT = sb.tile([128, T], BF16)
        # persistent group buffers
        qkall = sb.tile([128, NT, 256], BF16)   # roped inputs: q|k per tile
        qkrall = sb.tile([128, NT, 256], BF16)  # normed+roped q|k per tile
        vball = sb.tile([128, NT, 130], BF16)   # [vA|1|vB|1] per tile
        lsb = sb.tile([66, 8, 512], F32R)
        const_dmas.append(
            nc.gpsimd.dma_start(
                out=lsb[65:66, :, :], in_=d_lsbi.rearrange("o (n c) -> o n c", n=8)
            )
        )
        yts = sb.tile([128, T], BF16)

        # one-time init of the ones columns of vball (cols 64 and 129)
        nc.gpsimd.memset(
            vball[:].rearrange("p t (s c) -> p t s c", s=2)[:, :, :, 64:65], 1.0
        )

        # ---------------- emission helpers ----------------
        from concourse.tile import add_dep_helper

        first_mm = [None]  # tile-0 last matmul, for const-DMA deferral
        xt0_dma = [None]

        def emit_qkv_tile(ti):
            xt = sb_x.tile([128, 8, 128], BF16, tag="xt", name=f"xt{ti}")
            nc.sync.dma_start(out=xt[:, 0:4, :], in_=d_xtb[ti, :, 0:4, :])
            dma = nc.sync.dma_start(out=xt[:, 4:8, :], in_=d_xtb[ti, :, 4:8, :])
            if ti == 0:
                xt0_dma[0] = dma
                wqkv_rest[0] = nc.sync.dma_start(
                    out=wqkv[:, 2:8, :], in_=d_wqkv[:, 2:8, :]
                )
            psq = ps.tile([128, 384], F32, tag="qkvtr", name=f"psq{ti}")
            for i in range(8):
                mm = nc.tensor.matmul(
                    psq[:], xt[:, i, :], wqkv[:, i, :],
                    start=(i == 0), stop=(i == 7),
                )
            if ti == 0:
                first_mm[0] = mm
                for cd in const_dmas:
                    add_dep_helper(cd.ins, mm.ins, True, "defer const DMA")
                for cd in early_dmas:
                    add_dep_helper(cd.ins, xt0_dma[0].ins, True, "defer early DMA")
            # evacuate q,k (cast to bf16); blend v from PSUM on DVE
            nc.scalar.copy(qkall[:, ti, :], psq[:, 0:256])
            nc.vector.tensor_tensor(
                out=vball[:, ti, :].rearrange("p (s c) -> p s c", s=2)[:, :, 0:64],
                in0=psq[:, 256:384].rearrange("p (s c) -> p s c", s=2),
                in1=vi_t[:, ti, :].rearrange("p (s c) -> p s c", s=2),
                op=ALU.add,
            )

        def emit_chain(g):
            # group slice: tiles 4g..4g+3
            gsl = qkall[:, 4 * g : 4 * g + 4, :]
            # stats: Square (ACT) + one segmented reduce (DVE)
            sqt = sb_w1.tile([128, 1024], F32, tag="sqt", name=f"sqt{g}")
            nc.scalar.activation(
                sqt[:].rearrange("p (t c) -> p t c", t=4), gsl, AF.Square
            )
            nc.vector.tensor_reduce(
                stats[:, 16 * g : 16 * g + 16],
                sqt[:].rearrange("p (s c) -> p s c", s=16),
                axis=AX.X, op=ALU.add,
            )
            # batched rsqrt for tiles 4g..4g+3 (DVE bit-trick + 2 Newton iters)
            gg = 16 * g
            rs = rbuf[:, gg : gg + 16]
            zt = sb_w2.tile([128, 16], F32, tag="zt", name=f"zt{g}")
            nt1 = sb_w2.tile([128, 16], F32, tag="nt1", name=f"nt1{g}")
            nc.vector.tensor_scalar(
                out=zt[:], in0=stats[:, gg : gg + 16], scalar1=1.0 / 64.0,
                scalar2=RMS_EPS, op0=ALU.mult, op1=ALU.add,
            )
            nc.vector.tensor_scalar(
                out=nt1[:].bitcast(I32), in0=zt[:].bitcast(I32), scalar1=1,
                scalar2=0xFFFFFFFF, op0=ALU.logical_shift_right,
                op1=ALU.bitwise_xor,
            )
            nc.vector.tensor_scalar(
                out=rs.bitcast(I32), in0=nt1[:].bitcast(I32),
                scalar1=0x5F3759E0, scalar2=None, op0=ALU.add,
            )
            for _ in range(2):
                nc.vector.tensor_tensor(out=nt1[:], in0=rs, in1=rs, op=ALU.mult)
                nc.vector.tensor_tensor(out=nt1[:], in0=nt1[:], in1=zt[:], op=ALU.mult)
                nc.vector.tensor_scalar(
                    out=nt1[:], in0=nt1[:], scalar1=-0.5, scalar2=1.5,
                    op0=ALU.mult, op1=ALU.add,
                )
                nc.vector.tensor_tensor(out=rs, in0=rs, in1=nt1[:], op=ALU.mult)
            # fold 0.125 into the k columns of rbuf (cols 4t+2, 4t+3)
            kv = rbuf[:, gg : gg + 16].rearrange("p (t c) -> p t c", c=4)[:, :, 2:4]
            nc.vector.tensor_scalar_mul(kv, kv, 0.125)
            # bf16 copy of the scales for the bf16 norm-apply multiply
            nc.vector.tensor_copy(rbufb[:, gg : gg + 16], rbuf[:, gg : gg + 16])
            # rope, batched over the 4-tile group:
            #   tcb = qk * ccat ; tsb = swap(qk) * scat ; qkr = (tcb+tsb)*rs
            tcb = sb_w2.tile([128, 1024], BF16, tag="tcb", name=f"tcb{g}")
            nc.vector.tensor_tensor(
                out=tcb[:].rearrange("p (t s c) -> p t s c", t=4, s=4),
                in0=gsl.rearrange("p t (s c) -> p t s c", s=4),
                in1=cc_t[:, 4 * g : 4 * g + 4, :].unsqueeze(2)
                .broadcast_to((128, 4, 4, 64)),
                op=ALU.mult,
            )
            tsb = sb_w2.tile([128, 1024], BF16, tag="tsb", name=f"tsb{g}")
            ts4 = tsb[:].rearrange("p (t s h c) -> p t s h c", t=4, s=4, h=2)
            qk4 = gsl.rearrange("p t (s h c) -> p t s h c", s=4, h=2)
            nc.gpsimd.tensor_tensor(
                out=ts4[:, :, :, 0, :].rearrange("p t s c -> p (t s) c"),
                in0=qk4[:, :, :, 1, :].rearrange("p t s c -> p (t s) c"),
                in1=sc_t[:, 4 * g : 4 * g + 4, 0:32].unsqueeze(2)
                .broadcast_to((128, 4, 4, 32)),
                op=ALU.mult,
            )
            nc.gpsimd.tensor_tensor(
                out=ts4[:, :, :, 1, :].rearrange("p t s c -> p (t s) c"),
                in0=qk4[:, :, :, 0, :].rearrange("p t s c -> p (t s) c"),
                in1=sc_t[:, 4 * g : 4 * g + 4, 32:64].unsqueeze(2)
                .broadcast_to((128, 4, 4, 32)),
                op=ALU.mult,
            )
            nc.gpsimd.tensor_tensor(
                out=tcb[:], in0=tcb[:], in1=tsb[:], op=ALU.add
            )
            nc.vector.tensor_tensor(
                out=qkrall[:, 4 * g : 4 * g + 4, :]
                .rearrange("p t (s c) -> p (t s) c", s=4),
                in0=tcb[:].rearrange("p (m c) -> p m c", m=16),
                in1=rbufb[:, gg : gg + 16].unsqueeze(2)
                .broadcast_to((128, 16, 64)),
                op=ALU.mult,
            )

        def emit_transposes(g):
            for tj in range(4 * g, 4 * g + 4):
                for which, dst in ((0, qT), (1, kT)):
                    ptr = ps.tile(
                        [128, 128], BF16, tag="qkvtr", name=f"tr{tj}_{which}"
                    )
                    nc.tensor.transpose(
                        ptr[:], qkrall[:, tj, 128 * which : 128 * which + 128],
                        idn[:],
                    )
                    nc.vector.tensor_copy(
                        dst[:, 128 * tj : 128 * (tj + 1)], ptr[:]
                    )

        def emit_attention(ci, fillers=()):
            fillers = list(fillers)
            yt_h = []
            for h in range(2):
                kj_max = 4 * ci + 4
                yt = ps.tile([128, 512], F32, tag="ytmo", name=f"yt{ci}_{h}")
                yt_h.append(yt)
                for dd in range(kj_max // 2):
                    st = ps.tile([128, 1024], F32, tag="st", name=f"st{ci}_{h}_{dd}")
                    et = sb_e.tile([128, 1024], BF16, tag="et", name=f"et{ci}_{h}_{dd}")
                    qs = {}
                    for j2 in range(2):
                        kj = 2 * dd + j2
                        qs[j2] = 128 * (kj - 4 * ci) if kj >= 4 * ci else 0
                        nc.tensor.matmul(
                            st[:, 512 * j2 + qs[j2] : 512 * (j2 + 1)],
                            kT[64 * h : 64 * h + 64, 128 * kj : 128 * (kj + 1)],
                            qT[64 * h : 64 * h + 64, 512 * ci + qs[j2] : 512 * (ci + 1)],
                            start=True, stop=True,
                        )
                    nc.scalar.activation(et[:], st[:], AF.Exp)
                    for j2 in range(2):
                        kj = 2 * dd + j2
                        if kj >= 4 * ci:  # diagonal: tri-mask the block
                            blk = et[:, 512 * j2 + qs[j2] : 512 * j2 + qs[j2] + 128]
                            nc.vector.tensor_tensor(
                                out=blk, in0=blk, in1=tri[:], op=ALU.mult
                            )
                    for j2 in range(2):
                        kj = 2 * dd + j2
                        nc.tensor.matmul(
                            yt[0:65, qs[j2] : 512],
                            vball[:, kj, 65 * h : 65 * h + 65],
                            et[:, 512 * j2 + qs[j2] : 512 * (j2 + 1)],
                            start=(kj == 0), stop=(kj == kj_max - 1 and j2 == 1),
                        )
                    if fillers:
                        fillers.pop(0)()
            for f in fillers:
                f()
            return yt_h

        def emit_scale_outproj(ci, yt_h):
            mb = ps.tile([128, 1024], F32, tag="st", name=f"mb{ci}")
            mbs = sb_w2.tile([64, 1024], F32, tag="mbs", name=f"mbs{ci}")
            for h in range(2):
                nc.vector.tensor_copy(
                    lsb[64:65, 4 * h + ci, :], yt_h[h][64:65, 0:512]
                )
                nc.tensor.matmul(
                    mb[0:64, 512 * h : 512 * h + 512],
                    onp[64:66, 64 * h : 64 * h + 64],
                    lsb[64:66, 4 * h + ci, :],
                    start=True, stop=True,
                )
                nc.vector.reciprocal_approx_fast(
                    out=mbs[0:64, 512 * h : 512 * h + 512],
                    in_=mb[0:64, 512 * h : 512 * h + 512],
                )
                if h == 0:
                    nc.vector.tensor_tensor(
                        out=yts[0:64, 512 * ci : 512 * (ci + 1)],
                        in0=yt_h[h][0:64, 0:512],
                        in1=mbs[0:64, 512 * h : 512 * h + 512],
                        op=ALU.mult,
                    )
                else:
                    yts1 = sb_w2.tile([64, 512], BF16, tag="yts1", name=f"yts1_{ci}")
                    nc.vector.tensor_tensor(
                        out=yts1[:],
                        in0=yt_h[h][0:64, 0:512],
                        in1=mbs[0:64, 512 * h : 512 * h + 512],
                        op=ALU.mult,
                    )
                    nc.gpsimd.dma_start(
                        out=yts[64:128, 512 * ci : 512 * (ci + 1)], in_=yts1[:]
                    )
            for jt in range(8):
                pso = ps.tile([128, 512], F32, tag="ytmo", name=f"pso{ci}_{jt}")
                nc.tensor.matmul(
                    pso[:],
                    wo[:, 128 * jt : 128 * (jt + 1)],
                    yts[:, 512 * ci : 512 * (ci + 1)],
                    start=True, stop=True,
                )
                outsb = sb_o.tile([128, 512], BF16, tag="outsb", name=f"osb{ci}_{jt}")
                if jt % 2 == 0:
                    nc.vector.tensor_copy(outsb[:], pso[:])
                else:
                    nc.scalar.copy(outsb[:], pso[:])
                nc.sync.dma_start(
                    out=d_out[128 * jt : 128 * (jt + 1), 512 * ci : 512 * (ci + 1)],
                    in_=outsb[:],
                )

        # ---------------- HAM warm-up: ~3.5us of junk matmuls --------------
        wz = sb.tile([128, 512], BF16)
        nc.gpsimd.memset(wz[:], 0.0)
        pwz = ps.tile([128, 512], F32, tag="ytmo", name="pwz")
        for _w in range(14):
            nc.tensor.matmul(
                pwz[:, 0:256], wz[:, 0:128], wz[:, 0:256], start=True, stop=True
            )

        junk_n = [0]

        def emit_junk():
            # keep the HAM busy-window fed during ACT-bound attention spans
            junk_n[0] += 1
            pj = ps.tile([128, 384], F32, tag="qkvtr", name=f"junk{junk_n[0]}")
            for _ in range(4):
                nc.tensor.matmul(
                    pj[:], wz[:, 0:128], wz[:, 0:384], start=True, stop=True
                )

        # ---------------- interleaved emission ----------------
        # Software pipeline: QKV tiles for group ci+2 are interleaved into the
        # attention(ci) duo stream, so PE keeps streaming while ACT runs exps;
        # for the late (filler-less) phases junk matmuls keep the HAM warm.
        for ti in range(8):
            emit_qkv_tile(ti)
        emit_chain(0)
        emit_transposes(0)
        for ci in range(4):
            if ci < 2:
                fill = [
                    (lambda tj: (lambda: emit_qkv_tile(tj)))(tj)
                    for tj in range(4 * ci + 8, 4 * ci + 12)
                ]
            else:
                fill = [emit_junk] * (8 * ci + 4)
            yt_h = emit_attention(ci, fill)
            emit_scale_outproj(ci, yt_h)
            if ci < 3:
                emit_chain(ci + 1)
                emit_transposes(ci + 1)

    nc.compile()
    return nc


_NC = None


def _rope_tables():
    inv = (1.0 / 10000.0) ** (np.arange(0, HD, 2, dtype=np.float64) / HD)
    t = np.arange(T, dtype=np.float64)
    f = np.outer(t, inv)  # (T, 32)
    cc = np.concatenate([np.cos(f), np.cos(f)], axis=1).astype(np.float32)
    sc = np.concatenate([np.sin(f), -np.sin(f)], axis=1).astype(np.float32)
    return cc, sc


def kernel(x, vi, Wq, Wk, Wv, Wo, lamb, sink_weights):
    global _NC
    x = np.asarray(x, dtype=np.float32)
    vi = np.asarray(vi, dtype=np.float32)
    Wq = np.asarray(Wq, dtype=np.float32)
    Wk = np.asarray(Wk, dtype=np.float32)
    Wv = np.asarray(Wv, dtype=np.float32)
    Wo = np.asarray(Wo, dtype=np.float32)
    lam = float(np.asarray(lamb).reshape(-1)[0])
    sink = np.asarray(sink_weights, dtype=np.float32).reshape(-1)

    if _NC is None:
        _NC = _build_program()

    x0T = x[0].T  # (D, T)
    xtb = np.ascontiguousarray(
        x0T.reshape(8, 128, NT, 128).transpose(2, 1, 0, 3)
    ).astype(BF)  # (NT, p, i, c): xtb[ti, p, n, c] = xT[128n+p, 128ti+c]
    cc, sc = _rope_tables()
    ccb = np.ascontiguousarray(cc.reshape(NT, 128, 64).transpose(1, 0, 2)).astype(BF)
    scb = np.ascontiguousarray(sc.reshape(NT, 128, 64).transpose(1, 0, 2)).astype(BF)
    tri = (np.arange(128)[None, :] >= np.arange(128)[:, None]).astype(BF)
    idn = np.eye(128, dtype=np.float32).astype(BF)
    lsbi = np.ones((1, 4096), np.float32)

    in_maps = []
    for c in range(8):
        lo = 128 * c
        wqkv = np.concatenate(
            [
                Wq[lo : lo + 128].T,
                Wk[lo : lo + 128].T,
                (1.0 - lam) * Wv[lo : lo + 128].T,
            ],
            axis=1,
        )  # (D, 384)
        wqkv = np.ascontiguousarray(
            wqkv.reshape(8, 128, 384).transpose(1, 0, 2)
        ).astype(BF)
        onp = np.zeros((66, 128), np.float32)
        onp[64, :] = 1.0
        onp[65, 0:64] = np.exp(sink[2 * c])
        onp[65, 64:128] = np.exp(sink[2 * c + 1])
        in_maps.append(
            {
                "xtb": xtb,
                "wqkv": wqkv,
                "vis": np.ascontiguousarray(
                    (lam * vi[0][:, lo : lo + 128]).reshape(NT, 128, 128).transpose(1, 0, 2)
                ).astype(BF),
                "cc": ccb,
                "sc": scb,
                "wo": np.ascontiguousarray(Wo[:, lo : lo + 128].T).astype(BF),
                "idn": idn,
                "tri": tri,
                "onp": onp,
                "lsbi": lsbi,
            }
        )

    global _trace_in_maps
    _trace_in_maps = in_maps
    res = None
    for attempt in range(3):
        try:
            res = run_bass_kernel_spmd(_NC, in_maps, list(range(8)))
            break
        except Exception:
            # transient NRT_EXEC_UNIT_UNRECOVERABLE flakes have been seen on
            # the first execute after a fresh compile; retry
            if attempt == 2:
                raise
    outT = np.zeros((D, T), np.float64)
    for c in range(8):
        outT += np.asarray(res.results[c]["out"], dtype=np.float32)
    return np.ascontiguousarray(outT.T).astype(np.float32).reshape(1, T, D)


# revision 12
# speedup vs baseline: 1.0759x; 1.0759x over previous
"""Trainium2 Bass kernel for nn_CausalSelfAttention (B=1, T=2048, D=1024, H=16).

Sharding: 2 heads per core across 8 cores (tensor parallel). Wq/Wk/Wv
column-sharded by head, attention fully local, Wo row-sharded; host sums the
8 partial outputs (the all-reduce of the unshard step).

v3: bf16 matmul datapaths; rope/stats elementwise batched per 4-tile group
over persistent SBUF buffers (fewer, bigger DVE/GPSIMD instructions);
v-blend on DVE direct from PSUM; ones columns initialized once; longer HAM
warmup so the QKV phase starts at 2.4 GHz.

Per-core pipeline:
  P1  fused QKV: psum[t,384] = sum_i xT_blk.T @ [WqT|WkT|(1-l)WvT]; evac q,k
      (bf16) to qkall, v-blend (+lam*vi) from PSUM into vball.
  P2  per 4-tile group: RMS stats (Square+reduce), rsqrt bit-trick, RoPE via
      concat-table trick with sliced swap-reads, scale -> qkrall.
  P3  PE-transpose roped q,k -> qT,kT (d-major, bf16).
  P4  per (ci, head): ST[tk,tq] = kT_slice.T @ qT_chunk into 2-bank PSUM duos,
      one Exp per duo (bf16 out), tri-mask on diagonal blocks (DVE), matmul2
      YT[d|L,tq] with lhsT=[v|1] and rhs=E, e^sink accumulated via K=1 matmul
      so scale = sigmoid(lse-sink)/L = 1/(L + e^sink).
  P5  broadcast 1/(L+e^sink) across partitions via K=1 matmul + reciprocal,
      scale YT -> yts (bf16), out-proj per head (K=128), evacuate bf16, DMA.
"""

import sys

if "/opt/trn_rl_repo" not in sys.path:
    sys.path.insert(0, "/opt/trn_rl_repo")

import numpy as np
import ml_dtypes
from contextlib import ExitStack

from concourse import bacc, tile
from concourse import mybir
from concourse.bass_utils import run_bass_kernel_spmd

F32 = mybir.dt.float32
F32R = mybir.dt.float32r
BF16 = mybir.dt.bfloat16
I32 = mybir.dt.int32
AF = mybir.ActivationFunctionType
ALU = mybir.AluOpType
AX = mybir.AxisListType

T = 2048
D = 1024
HD = 64
NT = T // 128  # 16 t-tiles
RMS_EPS = float(np.finfo(np.float32).eps)
BF = ml_dtypes.bfloat16


def _build_program():
    nc = bacc.Bacc("TRN2", target_bir_lowering=False, debug=False, num_devices=8)

    d_xtb = nc.dram_tensor("xtb", [NT, 128, 8, 128], BF16, kind="ExternalInput").ap()
    d_wqkv = nc.dram_tensor("wqkv", [128, 8, 384], BF16, kind="ExternalInput").ap()
    d_vis = nc.dram_tensor("vis", [128, NT, 128], BF16, kind="ExternalInput").ap()
    d_cc = nc.dram_tensor("cc", [128, NT, 64], BF16, kind="ExternalInput").ap()
    d_sc = nc.dram_tensor("sc", [128, NT, 64], BF16, kind="ExternalInput").ap()
    d_wo = nc.dram_tensor("wo", [128, D], BF16, kind="ExternalInput").ap()
    d_idn = nc.dram_tensor("idn", [128, 128], BF16, kind="ExternalInput").ap()
    d_tri = nc.dram_tensor("tri", [128, 128], BF16, kind="ExternalInput").ap()
    d_onp = nc.dram_tensor("onp", [66, 128], F32R, kind="ExternalInput").ap()
    d_lsbi = nc.dram_tensor("lsbi", [1, 4096], F32R, kind="ExternalInput").ap()
    d_out = nc.dram_tensor("out", [D, T], BF16, kind="ExternalOutput").ap()

    with tile.TileContext(nc) as tc, ExitStack() as ctx:
        sb = ctx.enter_context(tc.tile_pool(name="sb", bufs=1))
        sb_x = ctx.enter_context(tc.tile_pool(name="sb_x", bufs=4))
        sb_w1 = ctx.enter_context(tc.tile_pool(name="sb_w1", bufs=3))
        sb_w2 = ctx.enter_context(tc.tile_pool(name="sb_w2", bufs=3))
        sb_e = ctx.enter_context(tc.tile_pool(name="sb_e", bufs=3))
        sb_o = ctx.enter_context(tc.tile_pool(name="sb_o", bufs=3))
        ps = ctx.enter_context(tc.tile_pool(name="ps", bufs=2, space="PSUM"))

        # weights first on the sync queue (needed by the first matmul);
        # other constants go via the gpsimd queue so they don't delay x.
        # Split so the first two k-chunks land before x tile 0, the rest after.
        wqkv = sb.tile([128, 8, 384], BF16)
        nc.sync.dma_start(out=wqkv[:, 0:2, :], in_=d_wqkv[:, 0:2, :])
        wqkv_rest = [None]
        vi_t = sb.tile([128, NT, 128], BF16)
        cc_t = sb.tile([128, NT, 64], BF16)
        sc_t = sb.tile([128, NT, 64], BF16)
        wo = sb.tile([128, D], BF16)
        const_dmas = []
        early_dmas = []
        early_dmas.append(nc.gpsimd.dma_start(out=vi_t[:], in_=d_vis[:]))
        early_dmas.append(nc.gpsimd.dma_start(out=cc_t[:], in_=d_cc[:]))
        early_dmas.append(nc.gpsimd.dma_start(out=sc_t[:], in_=d_sc[:]))
        const_dmas.append(nc.gpsimd.dma_start(out=wo[:], in_=d_wo[:]))
        idn = sb.tile([128, 128], BF16)
        early_dmas.append(nc.gpsimd.dma_start(out=idn[:], in_=d_idn[:]))
        tri = sb.tile([128, 128], BF16)
        const_dmas.append(nc.gpsimd.dma_start(out=tri[:], in_=d_tri[:]))
        onp = sb.tile([66, 128], F32R)
        const_dmas.append(nc.gpsimd.dma_start(out=onp[:], in_=d_onp[:]))

        stats = sb.tile([128, 64], F32)
        rbuf = sb.tile([128, 64], F32)
        rbufb = sb.tile([128, 64], BF16)
        qT = sb.tile([128, T], BF16)
        kT = sb.tile([128, T], BF16)
        # persistent group buffers
        qkall = sb.tile([128, NT, 256], BF16)   # roped inputs: q|k per tile
        qkrall = sb.tile([128, NT, 256], BF16)  # normed+roped q|k per tile
        vball = sb.tile([128, NT, 130], BF16)   # [vA|1|vB|1] per tile
        lsb = sb.tile([66, 8, 512], F32R)
        const_dmas.append(
            nc.gpsimd.dma_start(
                out=lsb[65:66, :, :], in_=d_lsbi.rearrange("o (n c) -> o n c", n=8)
            )
        )
        yts = sb.tile([128, T], BF16)

        # one-time init of the ones columns of vball (cols 64 and 129)
        nc.gpsimd.memset(
            vball[:].rearrange("p t (s c) -> p t s c", s=2)[:, :, :, 64:65], 1.0
        )

        # ---------------- emission helpers ----------------
        from concourse.tile import add_dep_helper

        first_mm = [None]  # tile-0 last matmul, for const-DMA deferral
        xt0_dma = [None]

        def emit_qkv_tile(ti):
            xt = sb_x.tile([128, 8, 128], BF16, tag="xt", name=f"xt{ti}")
            nc.sync.dma_start(out=xt[:, 0:4, :], in_=d_xtb[ti, :, 0:4, :])
            dma = nc.sync.dma_start(out=xt[:, 4:8, :], in_=d_xtb[ti, :, 4:8, :])
            if ti == 0:
                xt0_dma[0] = dma
                wqkv_rest[0] = nc.sync.dma_start(
                    out=wqkv[:, 2:8, :], in_=d_wqkv[:, 2:8, :]
                )
            psq = ps.tile([128, 384], F32, tag="qkvtr", name=f"psq{ti}")
            for i in range(8):
                mm = nc.tensor.matmul(
                    psq[:], xt[:, i, :], wqkv[:, i, :],
                    start=(i == 0), stop=(i == 7),
                )
            if ti == 0:
                first_mm[0] = mm
                for cd in const_dmas:
                    add_dep_helper(cd.ins, mm.ins, True, "defer const DMA")
                for cd in early_dmas:
                    add_dep_helper(cd.ins, xt0_dma[0].ins, True, "defer early DMA")
            # evacuate q,k (cast to bf16); blend v from PSUM on DVE
            nc.scalar.copy(qkall[:, ti, :], psq[:, 0:256])
            nc.vector.tensor_tensor(
                out=vball[:, ti, :].rearrange("p (s c) -> p s c", s=2)[:, :, 0:64],
                in0=psq[:, 256:384].rearrange("p (s c) -> p s c", s=2),
                in1=vi_t[:, ti, :].rearrange("p (s c) -> p s c", s=2),
                op=ALU.add,
            )

        def emit_chain(g):
            # group slice: tiles 4g..4g+3
            gsl = qkall[:, 4 * g : 4 * g + 4, :]
            # stats: Square (ACT) + one segmented reduce (DVE)
            sqt = sb_w1.tile([128, 1024], F32, tag="sqt", name=f"sqt{g}")
            nc.scalar.activation(
                sqt[:].rearrange("p (t c) -> p t c", t=4), gsl, AF.Square
            )
            nc.vector.tensor_reduce(
                stats[:, 16 * g : 16 * g + 16],
                sqt[:].rearrange("p (s c) -> p s c", s=16),
                axis=AX.X, op=ALU.add,
            )
            # batched rsqrt for tiles 4g..4g+3 (DVE bit-trick + 2 Newton iters)
            gg = 16 * g
            rs = rbuf[:, gg : gg + 16]
            zt = sb_w2.tile([128, 16], F32, tag="zt", name=f"zt{g}")
            nt1 = sb_w2.tile([128, 16], F32, tag="nt1", name=f"nt1{g}")
            nc.vector.tensor_scalar(
                out=zt[:], in0=stats[:, gg : gg + 16], scalar1=1.0 / 64.0,
                scalar2=RMS_EPS, op0=ALU.mult, op1=ALU.add,
            )
            nc.vector.tensor_scalar(
                out=nt1[:].bitcast(I32), in0=zt[:].bitcast(I32), scalar1=1,
                scalar2=0xFFFFFFFF, op0=ALU.logical_shift_right,
                op1=ALU.bitwise_xor,
            )
            nc.vector.tensor_scalar(
                out=rs.bitcast(I32), in0=nt1[:].bitcast(I32),
                scalar1=0x5F3759E0, scalar2=None, op0=ALU.add,
            )
            for _ in range(2):
                nc.vector.tensor_tensor(out=nt1[:], in0=rs, in1=rs, op=ALU.mult)
                nc.vector.tensor_tensor(out=nt1[:], in0=nt1[:], in1=zt[:], op=ALU.mult)
                nc.vector.tensor_scalar(
                    out=nt1[:], in0=nt1[:], scalar1=-0.5, scalar2=1.5,
                    op0=ALU.mult, op1=ALU.add,
                )
                nc.vector.tensor_tensor(out=rs, in0=rs, in1=nt1[:], op=ALU.mult)
            # fold 0.125 into the k columns of rbuf (cols 4t+2, 4t+3)
            kv = rbuf[:, gg : gg + 16].rearrange("p (t c) -> p t c", c=4)[:, :, 2:4]
            nc.vector.tensor_scalar_mul(kv, kv, 0.125)
            # bf16 copy of the scales for the bf16 norm-apply multiply
            nc.vector.tensor_copy(rbufb[:, gg : gg + 16], rbuf[:, gg : gg + 16])
            # rope, batched over the 4-tile group:
            #   tcb = qk * ccat ; tsb = swap(qk) * scat ; qkr = (tcb+tsb)*rs
            tcb = sb_w2.tile([128, 1024], BF16, tag="tcb", name=f"tcb{g}")
            nc.vector.tensor_tensor(
                out=tcb[:].rearrange("p (t s c) -> p t s c", t=4, s=4),
                in0=gsl.rearrange("p t (s c) -> p t s c", s=4),
                in1=cc_t[:, 4 * g : 4 * g + 4, :].unsqueeze(2)
                .broadcast_to((128, 4, 4, 64)),
                op=ALU.mult,
            )
            tsb = sb_w2.tile([128, 1024], BF16, tag="tsb", name=f"tsb{g}")
            ts4 = tsb[:].rearrange("p (t s h c) -> p t s h c", t=4, s=4, h=2)
            qk4 = gsl.rearrange("p t (s h c) -> p t s h c", s=4, h=2)
            nc.gpsimd.tensor_tensor(
                out=ts4[:, :, :, 0, :].rearrange("p t s c -> p (t s) c"),
                in0=qk4[:, :, :, 1, :].rearrange("p t s c -> p (t s) c"),
                in1=sc_t[:, 4 * g : 4 * g + 4, 0:32].unsqueeze(2)
                .broadcast_to((128, 4, 4, 32)),
                op=ALU.mult,
            )
            nc.gpsimd.tensor_tensor(
                out=ts4[:, :, :, 1, :].rearrange("p t s c -> p (t s) c"),
                in0=qk4[:, :, :, 0, :].rearrange("p t s c -> p (t s) c"),
                in1=sc_t[:, 4 * g : 4 * g + 4, 32:64].unsqueeze(2)
                .broadcast_to((128, 4, 4, 32)),
                op=ALU.mult,
            )
            nc.gpsimd.tensor_tensor(
                out=tcb[:], in0=tcb[:], in1=tsb[:], op=ALU.add
            )
            nc.vector.tensor_tensor(
                out=qkrall[:, 4 * g : 4 * g + 4, :]
                .rearrange("p t (s c) -> p (t s) c", s=4),
                in0=tcb[:].rearrange("p (m c) -> p m c", m=16),
                in1=rbufb[:, gg : gg + 16].unsqueeze(2)
                .broadcast_to((128, 16, 64)),
                op=ALU.mult,
            )

        def emit_transposes(g):
            for tj in range(4 * g, 4 * g + 4):
                for which, dst in ((0, qT), (1, kT)):
                    ptr = ps.tile(
                        [128, 128], BF16, tag="qkvtr", name=f"tr{tj}_{which}"
                    )
                    nc.tensor.transpose(
                        ptr[:], qkrall[:, tj, 128 * which : 128 * which + 128],
                        idn[:],
                    )
                    nc.vector.tensor_copy(
                        dst[:, 128 * tj : 128 * (tj + 1)], ptr[:]
                    )

        def emit_attention(ci, fillers=()):
            fillers = list(fillers)
            yt_h = []
            for h in range(2):
                kj_max = 4 * ci + 4
                yt = ps.tile([128, 512], F32, tag="ytmo", name=f"yt{ci}_{h}")
                yt_h.append(yt)
                for dd in range(kj_max // 2):
                    st = ps.tile([128, 1024], F32, tag="st", name=f"st{ci}_{h}_{dd}")
                    et = sb_e.tile([128, 1024], BF16, tag="et", name=f"et{ci}_{h}_{dd}")
                    qs = {}
                    for j2 in range(2):
                        kj = 2 * dd + j2
                        qs[j2] = 128 * (kj - 4 * ci) if kj >= 4 * ci else 0
                        nc.tensor.matmul(
                            st[:, 512 * j2 + qs[j2] : 512 * (j2 + 1)],
                            kT[64 * h : 64 * h + 64, 128 * kj : 128 * (kj + 1)],
                            qT[64 * h : 64 * h + 64, 512 * ci + qs[j2] : 512 * (ci + 1)],
                            start=True, stop=True,
                        )
                    nc.scalar.activation(et[:], st[:], AF.Exp)
                    for j2 in range(2):
                        kj = 2 * dd + j2
                        if kj >= 4 * ci:  # diagonal: tri-mask the block
                            blk = et[:, 512 * j2 + qs[j2] : 512 * j2 + qs[j2] + 128]
                            nc.vector.tensor_tensor(
                                out=blk, in0=blk, in1=tri[:], op=ALU.mult
                            )
                    for j2 in range(2):
                        kj = 2 * dd + j2
                        nc.tensor.matmul(
                            yt[0:65, qs[j2] : 512],
                            vball[:, kj, 65 * h : 65 * h + 65],
                            et[:, 512 * j2 + qs[j2] : 512 * (j2 + 1)],
                            start=(kj == 0), stop=(kj == kj_max - 1 and j2 == 1),
                        )
                    if fillers:
                        fillers.pop(0)()
            for f in fillers:
                f()
            return yt_h

        def emit_scale_outproj(ci, yt_h):
            mb = ps.tile([128, 1024], F32, tag="st", name=f"mb{ci}")
            mbs = sb_w2.tile([64, 1024], F32, tag="mbs", name=f"mbs{ci}")
            for h in range(2):
                nc.vector.tensor_copy(
                    lsb[64:65, 4 * h + ci, :], yt_h[h][64:65, 0:512]
                )
                nc.tensor.matmul(
                    mb[0:64, 512 * h : 512 * h + 512],
                    onp[64:66, 64 * h : 64 * h + 64],
                    lsb[64:66, 4 * h + ci, :],
                    start=True, stop=True,
                )
                nc.vector.reciprocal_approx_fast(
                    out=mbs[0:64, 512 * h : 512 * h + 512],
                    in_=mb[0:64, 512 * h : 512 * h + 512],
                )
                if h == 0:
                    nc.vector.tensor_tensor(
                        out=yts[0:64, 512 * ci : 512 * (ci + 1)],
                        in0=yt_h[h][0:64, 0:512],
                        in1=mbs[0:64, 512 * h : 512 * h + 512],
                        op=ALU.mult,
                    )
                else:
                    yts1 = sb_w2.tile([64, 512], BF16, tag="yts1", name=f"yts1_{ci}")
                    nc.vector.tensor_tensor(
                        out=yts1[:],
                        in0=yt_h[h][0:64, 0:512],
                        in1=mbs[0:64, 512 * h : 512 * h + 512],
                        op=ALU.mult,
                    )
                    nc.gpsimd.dma_start(
                        out=yts[64:128, 512 * ci : 512 * (ci + 1)], in_=yts1[:]
                    )
            for jt in range(8):
                pso = ps.tile([128, 512], F32, tag="ytmo", name=f"pso{ci}_{jt}")
                nc.tensor.matmul(
                    pso[:],
                    wo[:, 128 * jt : 128 * (jt + 1)],
                    yts[:, 512 * ci : 512 * (ci + 1)],
                    start=True, stop=True,
                )
                outsb = sb_o.tile([128, 512], BF16, tag="outsb", name=f"osb{ci}_{jt}")
                if jt % 2 == 0:
                    nc.vector.tensor_copy(outsb[:], pso[:])
                else:
                    nc.scalar.copy(outsb[:], pso[:])
                nc.sync.dma_start(
                    out=d_out[128 * jt : 128 * (jt + 1), 512 * ci : 512 * (ci + 1)],
                    in_=outsb[:],
                )

        # ---------------- HAM warm-up: ~3.5us of junk matmuls --------------
        wz = sb.tile([128, 512], BF16)
        nc.gpsimd.memset(wz[:], 0.0)
        pwz = ps.tile([128, 512], F32, tag="ytmo", name="pwz")
        for _w in range(14):
            nc.tensor.matmul(
                pwz[:, 0:256], wz[:, 0:128], wz[:, 0:256], start=True, stop=True
            )

        junk_n = [0]

        def emit_junk():
            # keep the HAM busy-window fed during ACT-bound attention spans
            junk_n[0] += 1
            pj = ps.tile([128, 384], F32, tag="qkvtr", name=f"junk{junk_n[0]}")
            for _ in range(4):
                nc.tensor.matmul(
                    pj[:], wz[:, 0:128], wz[:, 0:384], start=True, stop=True
                )

        # ---------------- interleaved emission ----------------
        # Software pipeline: QKV tiles for group ci+2 are interleaved into the
        # attention(ci) duo stream, so PE keeps streaming while ACT runs exps;
        # for the late (filler-less) phases junk matmuls keep the HAM warm.
        for ti in range(8):
            emit_qkv_tile(ti)
        emit_chain(0)
        emit_transposes(0)
        for ci in range(4):
            if ci < 2:
                fill = [
                    (lambda tj: (lambda: emit_qkv_tile(tj)))(tj)
                    for tj in range(4 * ci + 8, 4 * ci + 12)
                ]
            else:
                fill = []
            yt_h = emit_attention(ci, fill)
            emit_scale_outproj(ci, yt_h)
            if ci < 3:
                emit_chain(ci + 1)
                emit_transposes(ci + 1)

    nc.compile()
    return nc


_NC = None


def _rope_tables():
    inv = (1.0 / 10000.0) ** (np.arange(0, HD, 2, dtype=np.float64) / HD)
    t = np.arange(T, dtype=np.float64)
    f = np.outer(t, inv)  # (T, 32)
    cc = np.concatenate([np.cos(f), np.cos(f)], axis=1).astype(np.float32)
    sc = np.concatenate([np.sin(f), -np.sin(f)], axis=1).astype(np.float32)
    return cc, sc


def kernel(x, vi, Wq, Wk, Wv, Wo, lamb, sink_weights):
    global _NC
    x = np.asarray(x, dtype=np.float32)
    vi = np.asarray(vi, dtype=np.float32)
    Wq = np.asarray(Wq, dtype=np.float32)
    Wk = np.asarray(Wk, dtype=np.float32)
    Wv = np.asarray(Wv, dtype=np.float32)
    Wo = np.asarray(Wo, dtype=np.float32)
    lam = float(np.asarray(lamb).reshape(-1)[0])
    sink = np.asarray(sink_weights, dtype=np.float32).reshape(-1)

    if _NC is None:
        _NC = _build_program()

    x0T = x[0].T  # (D, T)
    xtb = np.ascontiguousarray(
        x0T.reshape(8, 128, NT, 128).transpose(2, 1, 0, 3)
    ).astype(BF)  # (NT, p, i, c): xtb[ti, p, n, c] = xT[128n+p, 128ti+c]
    cc, sc = _rope_tables()
    ccb = np.ascontiguousarray(cc.reshape(NT, 128, 64).transpose(1, 0, 2)).astype(BF)
    scb = np.ascontiguousarray(sc.reshape(NT, 128, 64).transpose(1, 0, 2)).astype(BF)
    tri = (np.arange(128)[None, :] >= np.arange(128)[:, None]).astype(BF)
    idn = np.eye(128, dtype=np.float32).astype(BF)
    lsbi = np.ones((1, 4096), np.float32)

    in_maps = []
    for c in range(8):
        lo = 128 * c
        wqkv = np.concatenate(
            [
                Wq[lo : lo + 128].T,
                Wk[lo : lo + 128].T,
                (1.0 - lam) * Wv[lo : lo + 128].T,
            ],
            axis=1,
        )  # (D, 384)
        wqkv = np.ascontiguousarray(
            wqkv.reshape(8, 128, 384).transpose(1, 0, 2)
        ).astype(BF)
        onp = np.zeros((66, 128), np.float32)
        onp[64, :] = 1.0
        onp[65, 0:64] = np.exp(sink[2 * c])
        onp[65, 64:128] = np.exp(sink[2 * c + 1])
        in_maps.append(
            {
                "xtb": xtb,
                "wqkv": wqkv,
                "vis": np.ascontiguousarray(
                    (lam * vi[0][:, lo : lo + 128]).reshape(NT, 128, 128).transpose(1, 0, 2)
                ).astype(BF),
                "cc": ccb,
                "sc": scb,
                "wo": np.ascontiguousarray(Wo[:, lo : lo + 128].T).astype(BF),
                "idn": idn,
                "tri": tri,
                "onp": onp,
                "lsbi": lsbi,
            }
        )

    global _trace_in_maps
    _trace_in_maps = in_maps
    res = None
    for attempt in range(3):
        try:
            res = run_bass_kernel_spmd(_NC, in_maps, list(range(8)))
            break
        except Exception:
            # transient NRT_EXEC_UNIT_UNRECOVERABLE flakes have been seen on
            # the first execute after a fresh compile; retry
            if attempt == 2:
                raise
    outT = np.zeros((D, T), np.float64)
    for c in range(8):
        outT += np.asarray(res.results[c]["out"], dtype=np.float32)
    return np.ascontiguousarray(outT.T).astype(np.float32).reshape(1, T, D)


# revision 13
# speedup vs baseline: 1.0942x; 1.0169x over previous
"""Trainium2 Bass kernel for nn_CausalSelfAttention (B=1, T=2048, D=1024, H=16).

Sharding: 2 heads per core across 8 cores (tensor parallel). Wq/Wk/Wv
column-sharded by head, attention fully local, Wo row-sharded; host sums the
8 partial outputs (the all-reduce of the unshard step).

v3: bf16 matmul datapaths; rope/stats elementwise batched per 4-tile group
over persistent SBUF buffers (fewer, bigger DVE/GPSIMD instructions);
v-blend on DVE direct from PSUM; ones columns initialized once; longer HAM
warmup so the QKV phase starts at 2.4 GHz.

Per-core pipeline:
  P1  fused QKV: psum[t,384] = sum_i xT_blk.T @ [WqT|WkT|(1-l)WvT]; evac q,k
      (bf16) to qkall, v-blend (+lam*vi) from PSUM into vball.
  P2  per 4-tile group: RMS stats (Square+reduce), rsqrt bit-trick, RoPE via
      concat-table trick with sliced swap-reads, scale -> qkrall.
  P3  PE-transpose roped q,k -> qT,kT (d-major, bf16).
  P4  per (ci, head): ST[tk,tq] = kT_slice.T @ qT_chunk into 2-bank PSUM duos,
      one Exp per duo (bf16 out), tri-mask on diagonal blocks (DVE), matmul2
      YT[d|L,tq] with lhsT=[v|1] and rhs=E, e^sink accumulated via K=1 matmul
      so scale = sigmoid(lse-sink)/L = 1/(L + e^sink).
  P5  broadcast 1/(L+e^sink) across partitions via K=1 matmul + reciprocal,
      scale YT -> yts (bf16), out-proj per head (K=128), evacuate bf16, DMA.
"""

import sys

if "/opt/trn_rl_repo" not in sys.path:
    sys.path.insert(0, "/opt/trn_rl_repo")

import numpy as np
import ml_dtypes
from contextlib import ExitStack

from concourse import bacc, tile
from concourse import mybir
from concourse.bass_utils import run_bass_kernel_spmd

F32 = mybir.dt.float32
F32R = mybir.dt.float32r
BF16 = mybir.dt.bfloat16
I32 = mybir.dt.int32
AF = mybir.ActivationFunctionType
ALU = mybir.AluOpType
AX = mybir.AxisListType

T = 2048
D = 1024
HD = 64
NT = T // 128  # 16 t-tiles
RMS_EPS = float(np.finfo(np.float32).eps)
BF = ml_dtypes.bfloat16


def _build_program():
    nc = bacc.Bacc("TRN2", target_bir_lowering=False, debug=False, num_devices=8)

    d_xtb = nc.dram_tensor("xtb", [NT, 128, 8, 128], BF16, kind="ExternalInput").ap()
    d_wqkv = nc.dram_tensor("wqkv", [128, 8, 384], BF16, kind="ExternalInput").ap()
    d_vis = nc.dram_tensor("vis", [128, NT, 128], BF16, kind="ExternalInput").ap()
    d_cc = nc.dram_tensor("cc", [128, NT, 64], BF16, kind="ExternalInput").ap()
    d_sc = nc.dram_tensor("sc", [128, NT, 64], BF16, kind="ExternalInput").ap()
    d_wo = nc.dram_tensor("wo", [128, D], BF16, kind="ExternalInput").ap()
    d_idn = nc.dram_tensor("idn", [128, 128], BF16, kind="ExternalInput").ap()
    d_tri = nc.dram_tensor("tri", [128, 128], BF16, kind="ExternalInput").ap()
    d_onp = nc.dram_tensor("onp", [66, 128], F32R, kind="ExternalInput").ap()
    d_lsbi = nc.dram_tensor("lsbi", [1, 4096], F32R, kind="ExternalInput").ap()
    d_out = nc.dram_tensor("out", [D, T], BF16, kind="ExternalOutput").ap()

    with tile.TileContext(nc) as tc, ExitStack() as ctx:
        sb = ctx.enter_context(tc.tile_pool(name="sb", bufs=1))
        sb_x = ctx.enter_context(tc.tile_pool(name="sb_x", bufs=4))
        sb_w1 = ctx.enter_context(tc.tile_pool(name="sb_w1", bufs=3))
        sb_w2 = ctx.enter_context(tc.tile_pool(name="sb_w2", bufs=3))
        sb_e = ctx.enter_context(tc.tile_pool(name="sb_e", bufs=3))
        sb_o = ctx.enter_context(tc.tile_pool(name="sb_o", bufs=3))
        ps = ctx.enter_context(tc.tile_pool(name="ps", bufs=2, space="PSUM"))

        # weights first on the sync queue (needed by the first matmul);
        # other constants go via the gpsimd queue so they don't delay x.
        # Split so the first two k-chunks land before x tile 0, the rest after.
        wqkv = sb.tile([128, 8, 384], BF16)
        nc.sync.dma_start(out=wqkv[:, 0:2, :], in_=d_wqkv[:, 0:2, :])
        wqkv_rest = [None]
        vi_t = sb.tile([128, NT, 128], BF16)
        cc_t = sb.tile([128, NT, 64], BF16)
        sc_t = sb.tile([128, NT, 64], BF16)
        wo = sb.tile([128, D], BF16)
        const_dmas = []
        early_dmas = []
        early_dmas.append(nc.gpsimd.dma_start(out=vi_t[:], in_=d_vis[:]))
        early_dmas.append(nc.gpsimd.dma_start(out=cc_t[:], in_=d_cc[:]))
        early_dmas.append(nc.gpsimd.dma_start(out=sc_t[:], in_=d_sc[:]))
        const_dmas.append(nc.gpsimd.dma_start(out=wo[:], in_=d_wo[:]))
        idn = sb.tile([128, 128], BF16)
        early_dmas.append(nc.gpsimd.dma_start(out=idn[:], in_=d_idn[:]))
        tri = sb.tile([128, 128], BF16)
        const_dmas.append(nc.gpsimd.dma_start(out=tri[:], in_=d_tri[:]))
        onp = sb.tile([66, 128], F32R)
        const_dmas.append(nc.gpsimd.dma_start(out=onp[:], in_=d_onp[:]))

        stats = sb.tile([128, 64], F32)
        rbuf = sb.tile([128, 64], F32)
        rbufb = sb.tile([128, 64], BF16)
        qT = sb.tile([128, T], BF16)
        kT = sb.tile([128, T], BF16)
        # persistent group buffers
        qkall = sb.tile([128, NT, 256], BF16)   # roped inputs: q|k per tile
        qkrall = sb.tile([128, NT, 256], BF16)  # normed+roped q|k per tile
        vball = sb.tile([128, NT, 130], BF16)   # [vA|1|vB|1] per tile
        lsb = sb.tile([66, 8, 512], F32R)
        const_dmas.append(
            nc.gpsimd.dma_start(
                out=lsb[65:66, :, :], in_=d_lsbi.rearrange("o (n c) -> o n c", n=8)
            )
        )
        yts = sb.tile([128, T], BF16)

        # one-time init of the ones columns of vball (cols 64 and 129)
        nc.gpsimd.memset(
            vball[:].rearrange("p t (s c) -> p t s c", s=2)[:, :, :, 64:65], 1.0
        )

        # ---------------- emission helpers ----------------
        from concourse.tile import add_dep_helper

        first_mm = [None]  # tile-0 last matmul, for const-DMA deferral
        xt0_dma = [None]

        def emit_qkv_tile(ti):
            xt = sb_x.tile([128, 8, 128], BF16, tag="xt", name=f"xt{ti}")
            nc.sync.dma_start(out=xt[:, 0:4, :], in_=d_xtb[ti, :, 0:4, :])
            dma = nc.sync.dma_start(out=xt[:, 4:8, :], in_=d_xtb[ti, :, 4:8, :])
            if ti == 0:
                xt0_dma[0] = dma
                wqkv_rest[0] = nc.sync.dma_start(
                    out=wqkv[:, 2:8, :], in_=d_wqkv[:, 2:8, :]
                )
            psq = ps.tile([128, 384], F32, tag="qkvtr", name=f"psq{ti}")
            for i in range(8):
                mm = nc.tensor.matmul(
                    psq[:], xt[:, i, :], wqkv[:, i, :],
                    start=(i == 0), stop=(i == 7),
                )
            if ti == 0:
                first_mm[0] = mm
                for cd in const_dmas:
                    add_dep_helper(cd.ins, mm.ins, True, "defer const DMA")
                for cd in early_dmas:
                    add_dep_helper(cd.ins, xt0_dma[0].ins, True, "defer early DMA")
            # evacuate q,k (cast to bf16); blend v from PSUM on DVE
            nc.scalar.copy(qkall[:, ti, :], psq[:, 0:256])
            nc.vector.tensor_tensor(
                out=vball[:, ti, :].rearrange("p (s c) -> p s c", s=2)[:, :, 0:64],
                in0=psq[:, 256:384].rearrange("p (s c) -> p s c", s=2),
                in1=vi_t[:, ti, :].rearrange("p (s c) -> p s c", s=2),
                op=ALU.add,
            )

        def emit_chain(g):
            # group slice: tiles 4g..4g+3
            gsl = qkall[:, 4 * g : 4 * g + 4, :]
            # stats: Square (ACT) + one segmented reduce (DVE)
            sqt = sb_w1.tile([128, 1024], F32, tag="sqt", name=f"sqt{g}")
            nc.scalar.activation(
                sqt[:].rearrange("p (t c) -> p t c", t=4), gsl, AF.Square
            )
            nc.vector.tensor_reduce(
                stats[:, 16 * g : 16 * g + 16],
                sqt[:].rearrange("p (s c) -> p s c", s=16),
                axis=AX.X, op=ALU.add,
            )
            # batched rsqrt for tiles 4g..4g+3 (DVE bit-trick + 2 Newton iters)
            gg = 16 * g
            rs = rbuf[:, gg : gg + 16]
            zt = sb_w2.tile([128, 16], F32, tag="zt", name=f"zt{g}")
            nt1 = sb_w2.tile([128, 16], F32, tag="nt1", name=f"nt1{g}")
            nc.vector.tensor_scalar(
                out=zt[:], in0=stats[:, gg : gg + 16], scalar1=1.0 / 64.0,
                scalar2=RMS_EPS, op0=ALU.mult, op1=ALU.add,
            )
            nc.vector.tensor_scalar(
                out=nt1[:].bitcast(I32), in0=zt[:].bitcast(I32), scalar1=1,
                scalar2=0xFFFFFFFF, op0=ALU.logical_shift_right,
                op1=ALU.bitwise_xor,
            )
            nc.vector.tensor_scalar(
                out=rs.bitcast(I32), in0=nt1[:].bitcast(I32),
                scalar1=0x5F3759E0, scalar2=None, op0=ALU.add,
            )
            for _ in range(2):
                nc.vector.tensor_tensor(out=nt1[:], in0=rs, in1=rs, op=ALU.mult)
                nc.vector.tensor_tensor(out=nt1[:], in0=nt1[:], in1=zt[:], op=ALU.mult)
                nc.vector.tensor_scalar(
                    out=nt1[:], in0=nt1[:], scalar1=-0.5, scalar2=1.5,
                    op0=ALU.mult, op1=ALU.add,
                )
                nc.vector.tensor_tensor(out=rs, in0=rs, in1=nt1[:], op=ALU.mult)
            # fold 0.125 into the k columns of rbuf (cols 4t+2, 4t+3)
            kv = rbuf[:, gg : gg + 16].rearrange("p (t c) -> p t c", c=4)[:, :, 2:4]
            nc.vector.tensor_scalar_mul(kv, kv, 0.125)
            # bf16 copy of the scales for the bf16 norm-apply multiply
            nc.vector.tensor_copy(rbufb[:, gg : gg + 16], rbuf[:, gg : gg + 16])
            # rope, batched over the 4-tile group:
            #   tcb = qk * ccat ; tsb = swap(qk) * scat ; qkr = (tcb+tsb)*rs
            tcb = sb_w2.tile([128, 1024], BF16, tag="tcb", name=f"tcb{g}")
            nc.vector.tensor_tensor(
                out=tcb[:].rearrange("p (t s c) -> p t s c", t=4, s=4),
                in0=gsl.rearrange("p t (s c) -> p t s c", s=4),
                in1=cc_t[:, 4 * g : 4 * g + 4, :].unsqueeze(2)
                .broadcast_to((128, 4, 4, 64)),
                op=ALU.mult,
            )
            tsb = sb_w2.tile([128, 1024], BF16, tag="tsb", name=f"tsb{g}")
            ts4 = tsb[:].rearrange("p (t s h c) -> p t s h c", t=4, s=4, h=2)
            qk4 = gsl.rearrange("p t (s h c) -> p t s h c", s=4, h=2)
            nc.gpsimd.tensor_tensor(
                out=ts4[:, :, :, 0, :].rearrange("p t s c -> p (t s) c"),
                in0=qk4[:, :, :, 1, :].rearrange("p t s c -> p (t s) c"),
                in1=sc_t[:, 4 * g : 4 * g + 4, 0:32].unsqueeze(2)
                .broadcast_to((128, 4, 4, 32)),
                op=ALU.mult,
            )
            nc.gpsimd.tensor_tensor(
                out=ts4[:, :, :, 1, :].rearrange("p t s c -> p (t s) c"),
                in0=qk4[:, :, :, 0, :].rearrange("p t s c -> p (t s) c"),
                in1=sc_t[:, 4 * g : 4 * g + 4, 32:64].unsqueeze(2)
                .broadcast_to((128, 4, 4, 32)),
                op=ALU.mult,
            )
            nc.gpsimd.tensor_tensor(
                out=tcb[:], in0=tcb[:], in1=tsb[:], op=ALU.add
            )
            nc.vector.tensor_tensor(
                out=qkrall[:, 4 * g : 4 * g + 4, :]
                .rearrange("p t (s c) -> p (t s) c", s=4),
                in0=tcb[:].rearrange("p (m c) -> p m c", m=16),
                in1=rbufb[:, gg : gg + 16].unsqueeze(2)
                .broadcast_to((128, 16, 64)),
                op=ALU.mult,
            )

        def emit_transposes(g):
            for tj in range(4 * g, 4 * g + 4):
                for which, dst in ((0, qT), (1, kT)):
                    ptr = ps.tile(
                        [128, 128], BF16, tag="qkvtr", name=f"tr{tj}_{which}"
                    )
                    nc.tensor.transpose(
                        ptr[:], qkrall[:, tj, 128 * which : 128 * which + 128],
                        idn[:],
                    )
                    nc.vector.tensor_copy(
                        dst[:, 128 * tj : 128 * (tj + 1)], ptr[:]
                    )

        def emit_attention(ci, fillers=()):
            # Software-pipelined duo stream: ST pair for duo i+1 is emitted
            # BEFORE duo i's YT pair, so the PE keeps streaming while ACT runs
            # exp(i) and exps go back-to-back instead of serializing with YTs.
            fillers = list(fillers)
            kj_max = 4 * ci + 4
            yt_h = [
                ps.tile([128, 512], F32, tag="ytmo", name=f"yt{ci}_{h}")
                for h in range(2)
            ]
            duos = [(h, dd) for h in range(2) for dd in range(kj_max // 2)]
            sts = {}

            def emit_st(idx):
                h, dd = duos[idx]
                st = ps.tile([128, 1024], F32, tag="st", name=f"st{ci}_{h}_{dd}")
                qs = {}
                for j2 in range(2):
                    kj = 2 * dd + j2
                    qs[j2] = 128 * (kj - 4 * ci) if kj >= 4 * ci else 0
                    nc.tensor.matmul(
                        st[:, 512 * j2 + qs[j2] : 512 * (j2 + 1)],
                        kT[64 * h : 64 * h + 64, 128 * kj : 128 * (kj + 1)],
                        qT[64 * h : 64 * h + 64, 512 * ci + qs[j2] : 512 * (ci + 1)],
                        start=True, stop=True,
                    )
                sts[idx] = (st, qs)

            emit_st(0)
            for idx, (h, dd) in enumerate(duos):
                if idx + 1 < len(duos):
                    emit_st(idx + 1)
                st, qs = sts.pop(idx)
                et = sb_e.tile([128, 1024], BF16, tag="et", name=f"et{ci}_{h}_{dd}")
                nc.scalar.activation(et[:], st[:], AF.Exp)
                for j2 in range(2):
                    kj = 2 * dd + j2
                    if kj >= 4 * ci:  # diagonal: tri-mask the block
                        blk = et[:, 512 * j2 + qs[j2] : 512 * j2 + qs[j2] + 128]
                        nc.vector.tensor_tensor(
                            out=blk, in0=blk, in1=tri[:], op=ALU.mult
                        )
                for j2 in range(2):
                    kj = 2 * dd + j2
                    nc.tensor.matmul(
                        yt_h[h][0:65, qs[j2] : 512],
                        vball[:, kj, 65 * h : 65 * h + 65],
                        et[:, 512 * j2 + qs[j2] : 512 * (j2 + 1)],
                        start=(kj == 0), stop=(kj == kj_max - 1 and j2 == 1),
                    )
                if fillers:
                    fillers.pop(0)()
            for f in fillers:
                f()
            return yt_h

        def emit_scale_outproj(ci, yt_h):
            mb = ps.tile([128, 1024], F32, tag="st", name=f"mb{ci}")
            mbs = sb_w2.tile([64, 1024], F32, tag="mbs", name=f"mbs{ci}")
            for h in range(2):
                nc.vector.tensor_copy(
                    lsb[64:65, 4 * h + ci, :], yt_h[h][64:65, 0:512]
                )
                nc.tensor.matmul(
                    mb[0:64, 512 * h : 512 * h + 512],
                    onp[64:66, 64 * h : 64 * h + 64],
                    lsb[64:66, 4 * h + ci, :],
                    start=True, stop=True,
                )
                nc.vector.reciprocal_approx_fast(
                    out=mbs[0:64, 512 * h : 512 * h + 512],
                    in_=mb[0:64, 512 * h : 512 * h + 512],
                )
                if h == 0:
                    nc.vector.tensor_tensor(
                        out=yts[0:64, 512 * ci : 512 * (ci + 1)],
                        in0=yt_h[h][0:64, 0:512],
                        in1=mbs[0:64, 512 * h : 512 * h + 512],
                        op=ALU.mult,
                    )
                else:
                    yts1 = sb_w2.tile([64, 512], BF16, tag="yts1", name=f"yts1_{ci}")
                    nc.vector.tensor_tensor(
                        out=yts1[:],
                        in0=yt_h[h][0:64, 0:512],
                        in1=mbs[0:64, 512 * h : 512 * h + 512],
                        op=ALU.mult,
                    )
                    nc.gpsimd.dma_start(
                        out=yts[64:128, 512 * ci : 512 * (ci + 1)], in_=yts1[:]
                    )
            for jt in range(8):
                pso = ps.tile([128, 512], F32, tag="ytmo", name=f"pso{ci}_{jt}")
                nc.tensor.matmul(
                    pso[:],
                    wo[:, 128 * jt : 128 * (jt + 1)],
                    yts[:, 512 * ci : 512 * (ci + 1)],
                    start=True, stop=True,
                )
                outsb = sb_o.tile([128, 512], BF16, tag="outsb", name=f"osb{ci}_{jt}")
                if jt % 2 == 0:
                    nc.vector.tensor_copy(outsb[:], pso[:])
                else:
                    nc.scalar.copy(outsb[:], pso[:])
                nc.sync.dma_start(
                    out=d_out[128 * jt : 128 * (jt + 1), 512 * ci : 512 * (ci + 1)],
                    in_=outsb[:],
                )

        # ---------------- HAM warm-up: ~3.5us of junk matmuls --------------
        wz = sb.tile([128, 512], BF16)
        nc.gpsimd.memset(wz[:], 0.0)
        pwz = ps.tile([128, 512], F32, tag="ytmo", name="pwz")
        for _w in range(14):
            nc.tensor.matmul(
                pwz[:, 0:256], wz[:, 0:128], wz[:, 0:256], start=True, stop=True
            )

        junk_n = [0]

        def emit_junk():
            # keep the HAM busy-window fed during ACT-bound attention spans
            junk_n[0] += 1
            pj = ps.tile([128, 384], F32, tag="qkvtr", name=f"junk{junk_n[0]}")
            for _ in range(4):
                nc.tensor.matmul(
                    pj[:], wz[:, 0:128], wz[:, 0:384], start=True, stop=True
                )

        # ---------------- interleaved emission ----------------
        # Software pipeline: QKV tiles for group ci+2 are interleaved into the
        # attention(ci) duo stream, so PE keeps streaming while ACT runs exps;
        # for the late (filler-less) phases junk matmuls keep the HAM warm.
        for ti in range(8):
            emit_qkv_tile(ti)
        emit_chain(0)
        emit_transposes(0)
        for ci in range(4):
            if ci < 2:
                fill = [
                    (lambda tj: (lambda: emit_qkv_tile(tj)))(tj)
                    for tj in range(4 * ci + 8, 4 * ci + 12)
                ]
            else:
                fill = []
            yt_h = emit_attention(ci, fill)
            emit_scale_outproj(ci, yt_h)
            if ci < 3:
                emit_chain(ci + 1)
                emit_transposes(ci + 1)

    nc.compile()
    return nc


_NC = None


def _rope_tables():
    inv = (1.0 / 10000.0) ** (np.arange(0, HD, 2, dtype=np.float64) / HD)
    t = np.arange(T, dtype=np.float64)
    f = np.outer(t, inv)  # (T, 32)
    cc = np.concatenate([np.cos(f), np.cos(f)], axis=1).astype(np.float32)
    sc = np.concatenate([np.sin(f), -np.sin(f)], axis=1).astype(np.float32)
    return cc, sc


def kernel(x, vi, Wq, Wk, Wv, Wo, lamb, sink_weights):
    global _NC
    x = np.asarray(x, dtype=np.float32)
    vi = np.asarray(vi, dtype=np.float32)
    Wq = np.asarray(Wq, dtype=np.float32)
    Wk = np.asarray(Wk, dtype=np.float32)
    Wv = np.asarray(Wv, dtype=np.float32)
    Wo = np.asarray(Wo, dtype=np.float32)
    lam = float(np.asarray(lamb).reshape(-1)[0])
    sink = np.asarray(sink_weights, dtype=np.float32).reshape(-1)

    if _NC is None:
        _NC = _build_program()

    x0T = x[0].T  # (D, T)
    xtb = np.ascontiguousarray(
        x0T.reshape(8, 128, NT, 128).transpose(2, 1, 0, 3)
    ).astype(BF)  # (NT, p, i, c): xtb[ti, p, n, c] = xT[128n+p, 128ti+c]
    cc, sc = _rope_tables()
    ccb = np.ascontiguousarray(cc.reshape(NT, 128, 64).transpose(1, 0, 2)).astype(BF)
    scb = np.ascontiguousarray(sc.reshape(NT, 128, 64).transpose(1, 0, 2)).astype(BF)
    tri = (np.arange(128)[None, :] >= np.arange(128)[:, None]).astype(BF)
    idn = np.eye(128, dtype=np.float32).astype(BF)
    lsbi = np.ones((1, 4096), np.float32)

    in_maps = []
    for c in range(8):
        lo = 128 * c
        wqkv = np.concatenate(
            [
                Wq[lo : lo + 128].T,
                Wk[lo : lo + 128].T,
                (1.0 - lam) * Wv[lo : lo + 128].T,
            ],
            axis=1,
        )  # (D, 384)
        wqkv = np.ascontiguousarray(
            wqkv.reshape(8, 128, 384).transpose(1, 0, 2)
        ).astype(BF)
        onp = np.zeros((66, 128), np.float32)
        onp[64, :] = 1.0
        onp[65, 0:64] = np.exp(sink[2 * c])
        onp[65, 64:128] = np.exp(sink[2 * c + 1])
        in_maps.append(
            {
                "xtb": xtb,
                "wqkv": wqkv,
                "vis": np.ascontiguousarray(
                    (lam * vi[0][:, lo : lo + 128]).reshape(NT, 128, 128).transpose(1, 0, 2)
                ).astype(BF),
                "cc": ccb,
                "sc": scb,
                "wo": np.ascontiguousarray(Wo[:, lo : lo + 128].T).astype(BF),
                "idn": idn,
                "tri": tri,
                "onp": onp,
                "lsbi": lsbi,
            }
        )

    global _trace_in_maps
    _trace_in_maps = in_maps
    res = None
    for attempt in range(3):
        try:
            res = run_bass_kernel_spmd(_NC, in_maps, list(range(8)))
            break
        except Exception:
            # transient NRT_EXEC_UNIT_UNRECOVERABLE flakes have been seen on
            # the first execute after a fresh compile; retry
            if attempt == 2:
                raise
    outT = np.zeros((D, T), np.float64)
    for c in range(8):
        outT += np.asarray(res.results[c]["out"], dtype=np.float32)
    return np.ascontiguousarray(outT.T).astype(np.float32).reshape(1, T, D)
